# revision 5
# baseline (speedup 1.0000x reference)
"""Trainium2 Bass kernel for the CCQC quantum-circuit classifier.

The whole circuit (one layer: RX/RZ/RX per qubit, then CPhase+RX ring) is a
fixed linear operator on the 1024-dim state vector.  On the host we fold all
40 gates into a single 1024x1024 complex matrix M (cheap numpy), so that for
a batch row xf:

    state_final = xn @ M            (xn = xf/||xf|| normalized on host)
    probs       = |state_final|^2
    outT        = signs^T probsT    (10, B) -- transposed; host transposes back

Device work per core (batch 512 of 4096), all fp16 operands (fp16 matmul is
1 cycle/row like fp32r but with half the HBM traffic; accuracy ~5e-4):
    RE^T = M_re^T xn^T, IM^T = M_im^T xn^T  (TensorE, K=1024 contraction)
    probsT = RE^2 + IM^2                    (ScalarE square -> fp16,
                                             VectorE add in fp16)
    outT  += sgn_jt^T probsT_jt             (TensorE fold per jt)
    outT -> SBUF -> DRAM (10, 512); host transposes/concats.

M is shipped as one interleaved tensor (JT, 128, KT, 256) with re in cols
0:128 and im in 128:256, one DMA per jt slab (jt0 split in two kt-halves so
the first matmul group can start as soon as 0.25 MB has landed).  Loads are
spread across BOTH HWDGE rings (Sync + Scalar engines) for parallel issue;
the store uses SWDGE (gpsimd).

The walrus build in this container allows AT MOST ONE sync-wait per
Matmult (its weight load is fused in) and per CTRL-class instruction.  All
cross-engine dependencies feeding the PE are therefore funneled through
single-wait gate instructions (1-column ldweights reads, which live on the
PE-engine proc so their observed clock carries over); sync=True same-engine
edges added with add_dep_helper pin the ordering without extra semaphores.
Pool buffer counts are chosen so no other WAR/WAW slot-reuse wait can pair
up with a producer wait on the same instruction, and the Tile kernel-tail
drain is monkeypatched into a chain of single-wait wait_ge instructions.
"""

import numpy as np

import concourse.bass as bass
import concourse.tile as tile
from concourse import mybir
from concourse.bass_utils import run_bass_kernel_spmd
from concourse.tile_rust import add_dep_helper

# The walrus build here accepts at most ONE sync wait per instruction, but
# Tile's kernel-tail emits a single Drain waiting on every proc's semaphore.
# Split that into a chain of single-wait pre-drains (one proc each); the
# final stock drain then finds everything already observed and gets no waits.
from concourse.tile_sem_assignment import tick_to_sem
from concourse.vector_clock import VectorClock  # noqa: F401 (repr eval below)


def _split_drain_and_barrier(self, tick_clock, wait_clock):
    ticks = eval(repr(tick_clock.global_clock)
                 .replace("VectorClock(", "").rstrip(")"))
    allocated = dict(wait_clock.sems.allocated())
    for p, t in enumerate(ticks):
        if t > 0 and p in allocated:
            self.nc.sync.wait_ge(allocated[p], tick_to_sem(t, p))
    self.nc.sync.drain()
    self.nc.all_engine_barrier()
    popped = self.nc._tile_sem_poison_stack.pop()
    assert popped is self._sem_poison
    self.nc.clear_and_free_semaphores(list(self.sems.allocated().values()))
    self.nc.all_engine_barrier()


tile.TileContext._drain_and_barrier = _split_drain_and_barrier

N_CORES = 8
N_QUBITS = 10
DIM = 1 << N_QUBITS          # 1024
B = 4096
BS = B // N_CORES            # 512 rows per core
KT = DIM // 128              # 8 contraction tiles
JT = DIM // 128              # 8 output-column tiles
H = BS // 2
XC = KT // 2

F16 = mybir.dt.float16
F32 = mybir.dt.float32


# ----------------------------------------------------------------- host math

def _build_circuit_matrix(weights):
    """M (DIM, DIM) complex128 with final_state_row = xf_row @ M."""
    w = np.asarray(weights, dtype=np.float64)
    M = np.eye(DIM, dtype=np.complex128)

    def apply_1q(state, U, wire):
        left = 1 << wire
        right = 1 << (N_QUBITS - 1 - wire)
        s = state.reshape(-1, left, 2, right)
        s0 = s[:, :, 0, :]
        s1 = s[:, :, 1, :]
        out = np.empty_like(s)
        out[:, :, 0, :] = U[0, 0] * s0 + U[0, 1] * s1
        out[:, :, 1, :] = U[1, 0] * s0 + U[1, 1] * s1
        return out.reshape(-1, DIM)

    def rx(t):
        c = np.cos(t / 2)
        s = -1j * np.sin(t / 2)
        return np.array([[c, s], [s, c]], dtype=np.complex128)

    def rz(t):
        return np.array(
            [[np.exp(-0.5j * t), 0], [0, np.exp(0.5j * t)]], dtype=np.complex128
        )

    d = 0
    for i in range(N_QUBITS):
        M = apply_1q(M, rx(w[d, i, 0]), i)
        M = apply_1q(M, rz(w[d, i, 1]), i)
        M = apply_1q(M, rx(w[d, i, 2]), i)
    j = 0
    idx = np.arange(DIM)
    for i in range(N_QUBITS):
        nj = (j + (N_QUBITS - 3)) % N_QUBITS
        hit = (
            (idx >> (N_QUBITS - 1 - j)) & (idx >> (N_QUBITS - 1 - nj)) & 1
        ).astype(bool)
        phase = np.where(hit, np.exp(1j * w[d, i, 3]), 1.0).astype(np.complex128)
        M = M * phase[None, :]
        M = apply_1q(M, rx(w[d, i, 4]), nj)
        j = nj
    return M


def _signs():
    """(DIM, N_QUBITS) fp32: PauliZ eigenvalue columns."""
    idx = np.arange(DIM)
    bits = (idx[:, None] >> (N_QUBITS - 1 - np.arange(N_QUBITS))[None, :]) & 1
    return (1.0 - 2.0 * bits).astype(np.float32)


def _pack_k_major(a):
    """(DIM, C) -> (128, KT, C): slab[p, t, c] = a[t*128 + p, c]."""
    c = a.shape[1]
    return np.ascontiguousarray(a.reshape(KT, 128, c).transpose(1, 0, 2))


def _pack_m(m):
    """(DIM, DIM) [k, j] -> (JT, 128, KT, 128): [jt][p, kt, j]."""
    a = m.reshape(KT, 128, JT, 128).transpose(2, 1, 0, 3)
    return np.ascontiguousarray(a)


# --------------------------------------------------------------- bass kernel

_CACHED_NC = None


def _build_bass():
    from contextlib import ExitStack

    nc = bass.Bass("TRN2")
    xt_d = nc.dram_tensor("xt", (128, KT, BS), F16, kind="ExternalInput")
    # interleaved slabs: cols 0:128 = re, 128:256 = im
    m_d = nc.dram_tensor("m", (JT, 128, KT, 256), F16, kind="ExternalInput")
    sgn_d = nc.dram_tensor("sgn", (128, KT, N_QUBITS), F16,
                           kind="ExternalInput")
    out_d = nc.dram_tensor("outT", (N_QUBITS, BS), F32, kind="ExternalOutput")

    with ExitStack() as es:
        tc = es.enter_context(tile.TileContext(nc))
        singles = es.enter_context(tc.tile_pool(name="singles", bufs=1))
        # one buffer per jt: kills every tmp-slot WAR/WAW dep
        tmps = es.enter_context(tc.tile_pool(name="tmps", bufs=JT))
        psum = es.enter_context(tc.tile_pool(name="psum", bufs=2, space="PSUM"))
        psum_w = es.enter_context(
            tc.tile_pool(name="psum_w", bufs=1, space="PSUM"))

        def pe_gate(ap):
            """Real PE-engine instruction (1-column fp16 ldweights) whose sole
            purpose is to carry one sync wait for `ap`'s producer; following
            matmuls then inherit the observed clock."""
            return nc.tensor.ldweights(weights=ap)

        def after(inst, gates):
            # sync=True same-engine edge: no semaphore, joins vector clocks,
            # pins scheduling order.
            for g in gates:
                add_dep_helper(inst.ins, g.ins, True, "order-after-gate")

        # ---- PE warmup ----
        # The PE clock is HAM-gated at 1.2 GHz until ~3.5us of sustained
        # activity.  The PE would otherwise idle during the xt/slab loads,
        # so burn that window on small dummy matmuls over a zeroed tile:
        # the real matmuls then ramp toward the full 2.4 GHz.
        zero_sb = singles.tile([128, 128], F16, tag="zero")
        nc.vector.memset(zero_sb, 0)
        warm_ps = psum_w.tile([128, 128], F32, tag="warm")
        N_WARM = 10
        for i in range(N_WARM):
            nc.tensor.matmul(
                warm_ps,
                lhsT=zero_sb[:],
                rhs=zero_sb[:],
                start=(i == 0),
                stop=(i == N_WARM - 1),
            )

        # ---- loads ----
        # Two HWDGE rings issue in parallel: Sync carries xt + odd jt slabs,
        # Scalar carries the (split) jt0 slab, sgn, and even jt slabs.
        xt_sb = singles.tile([128, KT, BS], F16, tag="xt")
        m_sb = singles.tile([128, JT, KT, 256], F16, tag="m")
        sgn_sb = singles.tile([128, KT, N_QUBITS], F16, tag="sgn")

        nc.sync.dma_start(out=xt_sb[:, 0:XC, :], in_=xt_d[:, 0:XC, :])
        nc.scalar.dma_start(out=m_sb[:, 0, 0:XC, :], in_=m_d[0, :, 0:XC, :])
        nc.sync.dma_start(out=xt_sb[:, XC:, :], in_=xt_d[:, XC:, :])
        nc.scalar.dma_start(out=m_sb[:, 0, XC:, :], in_=m_d[0, :, XC:, :])
        nc.scalar.dma_start(out=sgn_sb, in_=sgn_d[:])
        for jt in range(1, JT):
            eng = nc.sync if jt % 2 else nc.scalar
            eng.dma_start(out=m_sb[:, jt], in_=m_d[jt])

        probs_sb = singles.tile([128, JT, BS], F16, tag="probs")
        # separate tiles per half so each copy/store carries exactly one wait
        outT_sb_a = singles.tile([N_QUBITS, H], F32, tag="outT_a")
        outT_sb_b = singles.tile([N_QUBITS, H], F32, tag="outT_b")

        # PE observes the xt chunks and the two jt0 slab halves
        g_xt = [pe_gate(xt_sb[:, c * XC, 0:1]) for c in range(2)]
        g_m0b = pe_gate(m_sb[:, 0, XC, 0:1])
        g_sgn = pe_gate(sgn_sb[:, 0, 0:1])

        sq_hist = {"re": [], "im": []}
        outT_ps = psum_w.tile([N_QUBITS, BS], F32, tag="outT")

        def mm_group(part, jt, ps, b0, b1):
            off = 0 if part == "re" else 128
            gates = [pe_gate(m_sb[:, jt, 0, off:off + 1]), g_xt[0]]
            hist = sq_hist[part]
            if len(hist) >= 2:
                # psum slot last read by the square 2 allocations ago:
                # observing that square's output imports the needed ACT tick
                gates.append(pe_gate(hist[-2][:, 0:1]))
            for kt in range(KT):
                mm = nc.tensor.matmul(
                    ps,
                    lhsT=m_sb[:, jt, kt, off:off + 128],
                    rhs=xt_sb[:, kt, b0:b1],
                    start=(kt == 0),
                    stop=(kt == KT - 1),
                )
                if kt == 0:
                    after(mm, gates)
                elif kt == XC:
                    xgates = [g_xt[1]]
                    if jt == 0:
                        xgates.append(g_m0b)
                    after(mm, xgates)

        def postprocess(jt, ps_re_ap, ps_im_ap, b0, b1, o_start, o_stop):
            # squares on ACT (sole PSUM reader), sum on DVE (sole probs
            # writer); both in fp16 (half the write traffic, 2x DVE)
            nb = b1 - b0
            sq_re = tmps.tile([128, nb], F16, tag=f"sq_re{b0}")
            sq_im = tmps.tile([128, nb], F16, tag=f"sq_im{b0}")
            nc.scalar.activation(
                out=sq_re, in_=ps_re_ap,
                func=mybir.ActivationFunctionType.Square,
            )
            nc.scalar.activation(
                out=sq_im, in_=ps_im_ap,
                func=mybir.ActivationFunctionType.Square,
            )
            sq_hist["re"].append(sq_re)
            sq_hist["im"].append(sq_im)
            nc.vector.tensor_add(probs_sb[:, jt, b0:b1], sq_re, sq_im)

            # fold this jt's probs into the signs contraction right away:
            # signs stationary (10-col weight load is ~free), probs moving.
            mo = nc.tensor.matmul(
                outT_ps[:, b0:b1],
                lhsT=sgn_sb[:, jt, :],
                rhs=probs_sb[:, jt, b0:b1],
                start=o_start,
                stop=o_stop,
                skip_group_check=True,
            )
            if o_start:
                after(mo, [g_sgn])

        # NOTE: matmul start=True clears has_written for the WHOLE psum bank,
        # and cleared elements are overwritten (not accumulated) by the next
        # write - so exactly one start=True for the outT accumulation.
        for jt in range(JT - 1):
            ps_re = psum.tile([128, BS], F32, tag="ps_re")
            mm_group("re", jt, ps_re, 0, BS)
            ps_im = psum.tile([128, BS], F32, tag="ps_im")
            mm_group("im", jt, ps_im, 0, BS)
            postprocess(jt, ps_re[:, :], ps_im[:, :], 0, BS, jt == 0, False)

        # last jt: the im group (the end of the serial tail chain) runs as
        # two half-batch PSUM groups in separate banks, so the first half's
        # squares/adds/fold/copy/store pipeline under the second half's
        # matmuls
        jt = JT - 1
        ps_re = psum.tile([128, BS], F32, tag="ps_re")
        mm_group("re", jt, ps_re, 0, BS)
        ps_im_a = psum.tile([128, H], F32, tag="ps_im")
        mm_group("im", jt, ps_im_a, 0, H)
        postprocess(jt, ps_re[:, 0:H], ps_im_a[:, :], 0, H, False, False)
        ps_im_b = psum.tile([128, H], F32, tag="ps_im")
        mm_group("im", jt, ps_im_b, H, BS)
        # first half's result leaves the chip while the second half computes
        nc.vector.tensor_copy(out=outT_sb_a, in_=outT_ps[:, 0:H])
        nc.gpsimd.dma_start(out=out_d[:, 0:H], in_=outT_sb_a)
        postprocess(jt, ps_re[:, H:BS], ps_im_b[:, :], H, BS, False, True)
        nc.vector.tensor_copy(out=outT_sb_b, in_=outT_ps[:, H:BS])
        nc.gpsimd.dma_start(out=out_d[:, H:BS], in_=outT_sb_b)

    # Tile emits a same-proc sem wait on the second outT copy (DVE waiting on
    # its own tick semaphore for the first copy).  Same-engine program order
    # already guarantees it, and walrus allows only one wait per instruction:
    # drop any wait on the semaphore an instruction itself increments when
    # the waited value precedes the instruction's own tick.
    for blk in nc.m.functions[0].blocks:
        for inst in blk.instructions:
            si = getattr(inst, "sync_info", None)
            if not si or not si.on_wait or len(si.on_wait) <= 1:
                continue
            own_sems = {u.id for u in (si.on_update or [])
                        if u.update_mode == "sem-inc"}
            tick = inst.bass_scheduled_tick
            kept = [w for w in si.on_wait
                    if not (w.id in own_sems and tick is not None
                            and w.wait_value < tick)]
            assert kept, f"{inst.name}: all waits dropped"
            si.on_wait = kept

    return nc


def _get_nc():
    global _CACHED_NC
    if _CACHED_NC is None:
        _CACHED_NC = _build_bass()
    return _CACHED_NC


# ----------------------------------------------------------------- entrypoint

def kernel(x, weights, weights_1, weights_2, _trace=False):
    x = np.asarray(x, dtype=np.float32)
    xf = x.reshape(B, DIM)
    # normalize rows on the host (packing-time math): the device then skips
    # the reciprocal/divide entirely and the signs contraction is final
    xf = xf / np.sqrt(np.sum(xf * xf, axis=1, keepdims=True))

    M = _build_circuit_matrix(weights)
    mre_pack = _pack_m(M.real.astype(np.float32))  # (JT, 128, KT, 128)
    mim_pack = _pack_m(M.imag.astype(np.float32))
    m_pack = np.concatenate([mre_pack, mim_pack], axis=-1).astype(np.float16)
    sgn_pack = _pack_k_major(_signs()).astype(np.float16)

    in_maps = []
    for c in range(N_CORES):
        shard = xf[c * BS:(c + 1) * BS]              # (BS, DIM)
        xt = np.ascontiguousarray(shard.T)           # (DIM, BS)
        xt_pack = _pack_k_major(xt).astype(np.float16)  # (128, KT, BS)
        in_maps.append({
            "xt": xt_pack,
            "m": m_pack,
            "sgn": sgn_pack,
        })

    nc = _get_nc()
    res = run_bass_kernel_spmd(nc, in_maps, core_ids=list(range(N_CORES)),
                               trace=_trace)
    out = np.concatenate([r["outT"].T for r in res.results], axis=0)
    if _trace:
        kernel.last_exec_time_ns = res.exec_time_ns
        kernel.last_results = res
    return np.ascontiguousarray(out, dtype=np.float32)


# revision 6
# speedup vs baseline: 1.2612x; 1.2612x over previous
"""Trainium2 Bass kernel for the CCQC quantum-circuit classifier.

The whole circuit (one layer: RX/RZ/RX per qubit, then CPhase+RX ring) is a
fixed linear operator on the 1024-dim state vector.  On the host we fold all
40 gates into a single 1024x1024 complex matrix M (cheap numpy), so that for
a batch row xf:

    state_final = xn @ M            (xn = xf/||xf|| normalized on host)
    probs       = |state_final|^2
    outT        = signs^T probsT    (10, B) -- transposed; host transposes back

Device work per core (batch 512 of 4096), all fp16 operands (fp16 matmul is
1 cycle/row like fp32r but with half the HBM traffic; accuracy ~6e-4):
    RE^T = M_re^T xn^T, IM^T = M_im^T xn^T  (TensorE, K=1024 contraction)
    probsT = RE^2 + IM^2                    (ScalarE square -> fp16,
                                             VectorE add in fp16)
    outT  += sgn_jt^T probsT_jt             (TensorE fold, emitted one group
                                             late so its DVE wait never
                                             stalls the PE sequencer)
    outT -> SBUF -> DRAM (10, 512); host transposes/concats.

Loads are spread across BOTH HWDGE rings (Sync + Scalar engines): xt goes in
four kt-sliced chunks and the jt0 re-slab in two, so the first matmul group
starts as soon as ~0.3 MB has landed (the DMA queue has ~2us of startup
latency, so small leading chunks matter).  The store uses SWDGE (gpsimd).

The walrus build in this container allows AT MOST ONE sync-wait per
Matmult (its weight load is fused in) and per CTRL-class instruction.  All
cross-engine dependencies feeding the PE are therefore funneled through
single-wait gate instructions (1-column ldweights reads, which live on the
PE-engine proc so their observed clock carries over); sync=True same-engine
edges added with add_dep_helper pin the ordering without extra semaphores.
Pool buffer counts are chosen so no other WAR/WAW slot-reuse wait can pair
up with a producer wait on the same instruction, the Tile kernel-tail drain
is monkeypatched into a chain of single-wait wait_ge instructions, and a
post-pass strips Tile's redundant same-proc waits (program order already
guarantees them).
"""

import numpy as np

import concourse.bass as bass
import concourse.tile as tile
from concourse import mybir
from concourse.bass_utils import run_bass_kernel_spmd
from concourse.tile_rust import add_dep_helper

# The walrus build here accepts at most ONE sync wait per instruction, but
# Tile's kernel-tail emits a single Drain waiting on every proc's semaphore.
# Split that into a chain of single-wait pre-drains (one proc each); the
# final stock drain then finds everything already observed and gets no waits.
from concourse.tile_sem_assignment import tick_to_sem
from concourse.vector_clock import VectorClock  # noqa: F401 (repr eval below)


def _split_drain_and_barrier(self, tick_clock, wait_clock):
    ticks = eval(repr(tick_clock.global_clock)
                 .replace("VectorClock(", "").rstrip(")"))
    allocated = dict(wait_clock.sems.allocated())
    for p, t in enumerate(ticks):
        if t > 0 and p in allocated:
            self.nc.sync.wait_ge(allocated[p], tick_to_sem(t, p))
    self.nc.sync.drain()
    self.nc.all_engine_barrier()
    popped = self.nc._tile_sem_poison_stack.pop()
    assert popped is self._sem_poison
    self.nc.clear_and_free_semaphores(list(self.sems.allocated().values()))
    self.nc.all_engine_barrier()


tile.TileContext._drain_and_barrier = _split_drain_and_barrier

N_CORES = 8
N_QUBITS = 10
DIM = 1 << N_QUBITS          # 1024
B = 4096
BS = B // N_CORES            # 512 rows per core
KT = DIM // 128              # 8 contraction tiles
JT = DIM // 128              # 8 output-column tiles
H = BS // 2

F16 = mybir.dt.float16
F32 = mybir.dt.float32


# ----------------------------------------------------------------- host math

def _build_circuit_matrix(weights):
    """M (DIM, DIM) complex128 with final_state_row = xf_row @ M."""
    w = np.asarray(weights, dtype=np.float64)
    M = np.eye(DIM, dtype=np.complex128)

    def apply_1q(state, U, wire):
        left = 1 << wire
        right = 1 << (N_QUBITS - 1 - wire)
        s = state.reshape(-1, left, 2, right)
        s0 = s[:, :, 0, :]
        s1 = s[:, :, 1, :]
        out = np.empty_like(s)
        out[:, :, 0, :] = U[0, 0] * s0 + U[0, 1] * s1
        out[:, :, 1, :] = U[1, 0] * s0 + U[1, 1] * s1
        return out.reshape(-1, DIM)

    def rx(t):
        c = np.cos(t / 2)
        s = -1j * np.sin(t / 2)
        return np.array([[c, s], [s, c]], dtype=np.complex128)

    def rz(t):
        return np.array(
            [[np.exp(-0.5j * t), 0], [0, np.exp(0.5j * t)]], dtype=np.complex128
        )

    d = 0
    for i in range(N_QUBITS):
        M = apply_1q(M, rx(w[d, i, 0]), i)
        M = apply_1q(M, rz(w[d, i, 1]), i)
        M = apply_1q(M, rx(w[d, i, 2]), i)
    j = 0
    idx = np.arange(DIM)
    for i in range(N_QUBITS):
        nj = (j + (N_QUBITS - 3)) % N_QUBITS
        hit = (
            (idx >> (N_QUBITS - 1 - j)) & (idx >> (N_QUBITS - 1 - nj)) & 1
        ).astype(bool)
        phase = np.where(hit, np.exp(1j * w[d, i, 3]), 1.0).astype(np.complex128)
        M = M * phase[None, :]
        M = apply_1q(M, rx(w[d, i, 4]), nj)
        j = nj
    return M


def _signs():
    """(DIM, N_QUBITS) fp32: PauliZ eigenvalue columns."""
    idx = np.arange(DIM)
    bits = (idx[:, None] >> (N_QUBITS - 1 - np.arange(N_QUBITS))[None, :]) & 1
    return (1.0 - 2.0 * bits).astype(np.float32)


def _pack_k_major(a):
    """(DIM, C) -> (128, KT, C): slab[p, t, c] = a[t*128 + p, c]."""
    c = a.shape[1]
    return np.ascontiguousarray(a.reshape(KT, 128, c).transpose(1, 0, 2))


def _pack_m(m):
    """(DIM, DIM) [k, j] -> (JT, 128, KT, 128): [jt][p, kt, j]."""
    a = m.reshape(KT, 128, JT, 128).transpose(2, 1, 0, 3)
    return np.ascontiguousarray(a)


# --------------------------------------------------------------- bass kernel

_CACHED_NC = None


def _build_bass():
    from contextlib import ExitStack

    nc = bass.Bass("TRN2")
    xt_d = nc.dram_tensor("xt", (128, KT, BS), F16, kind="ExternalInput")
    mre_d = nc.dram_tensor("m_re", (JT, 128, KT, 128), F16,
                           kind="ExternalInput")
    mim_d = nc.dram_tensor("m_im", (JT, 128, KT, 128), F16,
                           kind="ExternalInput")
    sgn_d = nc.dram_tensor("sgn", (128, KT, N_QUBITS), F16,
                           kind="ExternalInput")
    out_d = nc.dram_tensor("outT", (N_QUBITS, BS), F32, kind="ExternalOutput")

    with ExitStack() as es:
        tc = es.enter_context(tile.TileContext(nc))
        singles = es.enter_context(tc.tile_pool(name="singles", bufs=1))
        # one buffer per jt: kills every tmp-slot WAR/WAW dep
        tmps = es.enter_context(tc.tile_pool(name="tmps", bufs=JT))
        psum = es.enter_context(tc.tile_pool(name="psum", bufs=3, space="PSUM"))
        psum_w = es.enter_context(
            tc.tile_pool(name="psum_w", bufs=1, space="PSUM"))

        def pe_gate(ap):
            """Real PE-engine instruction (1-column fp16 ldweights) whose sole
            purpose is to carry one sync wait for `ap`'s producer; following
            matmuls then inherit the observed clock."""
            return nc.tensor.ldweights(weights=ap)

        def after(inst, gates):
            # sync=True same-engine edge: no semaphore, joins vector clocks,
            # pins scheduling order.
            for g in gates:
                add_dep_helper(inst.ins, g.ins, True, "order-after-gate")

        # ---- PE warmup ----
        # The PE clock is HAM-gated at 1.2 GHz until ~3.5us of sustained
        # activity, and the first xt/slab chunks take until ~10us to land
        # (DMA queue startup).  Burn that window on small dummy matmuls so
        # the PE ramps toward 2.4 GHz under the real stream.
        zero_sb = singles.tile([128, 128], F16, tag="zero")
        nc.vector.memset(zero_sb, 0)
        warm_ps = psum_w.tile([128, 128], F32, tag="warm")
        N_WARM = 16
        for i in range(N_WARM):
            nc.tensor.matmul(
                warm_ps,
                lhsT=zero_sb[:],
                rhs=zero_sb[:],
                start=(i == 0),
                stop=(i == N_WARM - 1),
            )

        # ---- loads ----
        # Two HWDGE rings issue in parallel.  Leading chunks are small so the
        # first group can start ~10us in; later slabs stream far ahead of
        # consumption.
        xt_sb = singles.tile([128, KT, BS], F16, tag="xt")
        mre_sb = singles.tile([128, JT, KT, 128], F16, tag="mre")
        mim_sb = singles.tile([128, JT, KT, 128], F16, tag="mim")
        sgn_sb = singles.tile([128, KT, N_QUBITS], F16, tag="sgn")

        XN = 4                    # xt chunks
        XW = KT // XN             # kt per xt chunk
        for c in range(XN):
            nc.sync.dma_start(out=xt_sb[:, c * XW:(c + 1) * XW, :],
                              in_=xt_d[:, c * XW:(c + 1) * XW, :])
        nc.scalar.dma_start(out=mre_sb[:, 0, 0:2], in_=mre_d[0, :, 0:2])
        nc.scalar.dma_start(out=mre_sb[:, 0, 2:], in_=mre_d[0, :, 2:])
        nc.scalar.dma_start(out=mim_sb[:, 0], in_=mim_d[0])
        nc.scalar.dma_start(out=sgn_sb, in_=sgn_d[:])
        for jt in range(1, JT):
            eng = nc.scalar if jt % 2 else nc.sync
            eng.dma_start(out=mre_sb[:, jt], in_=mre_d[jt])
            eng.dma_start(out=mim_sb[:, jt], in_=mim_d[jt])

        probs_sb = singles.tile([128, JT, BS], F16, tag="probs")
        # separate tiles per half so each copy/store carries exactly one wait
        outT_sb_a = singles.tile([N_QUBITS, H], F32, tag="outT_a")
        outT_sb_b = singles.tile([N_QUBITS, H], F32, tag="outT_b")

        # PE observes the xt chunks and the split jt0 re-slab
        g_xt = [pe_gate(xt_sb[:, c * XW, 0:1]) for c in range(XN)]
        g_mre0b = pe_gate(mre_sb[:, 0, 2, 0:1])
        g_sgn = pe_gate(sgn_sb[:, 0, 0:1])

        sq_hist = {"re": [], "im": []}
        outT_ps = psum_w.tile([N_QUBITS, BS], F32, tag="outT")

        def mm_group(part, jt, ps, b0, b1):
            m_sb = mre_sb if part == "re" else mim_sb
            gates = [pe_gate(m_sb[:, jt, 0, 0:1]), g_xt[0]]
            hist = sq_hist[part]
            if len(hist) >= 3:
                # psum slot last read by the square 3 allocations ago:
                # observing that square's output imports the needed ACT tick
                gates.append(pe_gate(hist[-3][:, 0:1]))
            for kt in range(KT):
                mm = nc.tensor.matmul(
                    ps,
                    lhsT=m_sb[:, jt, kt, :],
                    rhs=xt_sb[:, kt, b0:b1],
                    start=(kt == 0),
                    stop=(kt == KT - 1),
                )
                if kt == 0:
                    after(mm, gates)
                elif kt % XW == 0:
                    xgates = [g_xt[kt // XW]]
                    if jt == 0 and part == "re" and kt == 2:
                        xgates.append(g_mre0b)
                    after(mm, xgates)
                elif jt == 0 and part == "re" and kt == 2:
                    after(mm, [g_mre0b])

        def squares(jt, ps_re_ap, ps_im_ap, b0, b1):
            # squares on ACT (sole PSUM reader), sum on DVE (sole probs
            # writer); both in fp16 (half the write traffic, 2x DVE)
            nb = b1 - b0
            sq_re = tmps.tile([128, nb], F16, tag=f"sq_re{b0}")
            sq_im = tmps.tile([128, nb], F16, tag=f"sq_im{b0}")
            nc.scalar.activation(
                out=sq_re, in_=ps_re_ap,
                func=mybir.ActivationFunctionType.Square,
            )
            nc.scalar.activation(
                out=sq_im, in_=ps_im_ap,
                func=mybir.ActivationFunctionType.Square,
            )
            sq_hist["re"].append(sq_re)
            sq_hist["im"].append(sq_im)
            nc.vector.tensor_add(probs_sb[:, jt, b0:b1], sq_re, sq_im)

        def fold(jt, b0, b1, o_start, o_stop):
            # fold jt's probs into the signs contraction: signs stationary
            # (10-col weight load is ~free), probs moving.  Emitted one
            # group AFTER its producer so the cross-engine DVE wait is
            # long-satisfied and never stalls the PE sequencer.
            mo = nc.tensor.matmul(
                outT_ps[:, b0:b1],
                lhsT=sgn_sb[:, jt, :],
                rhs=probs_sb[:, jt, b0:b1],
                start=o_start,
                stop=o_stop,
                skip_group_check=True,
            )
            if o_start:
                after(mo, [g_sgn])

        # NOTE: matmul start=True clears has_written for the WHOLE psum bank,
        # and cleared elements are overwritten (not accumulated) by the next
        # write - so exactly one start=True for the outT accumulation.
        for jt in range(JT - 1):
            ps_re = psum.tile([128, BS], F32, tag="ps_re")
            mm_group("re", jt, ps_re, 0, BS)
            if jt >= 1:
                fold(jt - 1, 0, BS, jt == 1, False)
            ps_im = psum.tile([128, BS], F32, tag="ps_im")
            mm_group("im", jt, ps_im, 0, BS)
            squares(jt, ps_re[:, :], ps_im[:, :], 0, BS)

        # last jt: the im group (the end of the serial tail chain) runs as
        # two half-batch PSUM groups in separate banks, so the first half's
        # squares/adds/fold/copy/store pipeline under the second half's
        # matmuls
        jt = JT - 1
        ps_re = psum.tile([128, BS], F32, tag="ps_re")
        mm_group("re", jt, ps_re, 0, BS)
        fold(jt - 1, 0, BS, False, False)
        ps_im_a = psum.tile([128, H], F32, tag="ps_im")
        mm_group("im", jt, ps_im_a, 0, H)
        squares(jt, ps_re[:, 0:H], ps_im_a[:, :], 0, H)
        ps_im_b = psum.tile([128, H], F32, tag="ps_im")
        mm_group("im", jt, ps_im_b, H, BS)
        fold(jt, 0, H, False, False)
        # first half's result leaves the chip while the second half computes
        nc.vector.tensor_copy(out=outT_sb_a, in_=outT_ps[:, 0:H])
        nc.gpsimd.dma_start(out=out_d[:, 0:H], in_=outT_sb_a)
        squares(jt, ps_re[:, H:BS], ps_im_b[:, :], H, BS)
        fold(jt, H, BS, False, True)
        nc.vector.tensor_copy(out=outT_sb_b, in_=outT_ps[:, H:BS])
        nc.gpsimd.dma_start(out=out_d[:, H:BS], in_=outT_sb_b)

    # Tile occasionally emits a same-proc sem wait (e.g. DVE waiting on its
    # own tick semaphore for an earlier DVE instruction).  Same-engine
    # program order already guarantees those, and walrus allows only one
    # wait per instruction: drop any wait on the semaphore an instruction
    # itself increments when the waited value precedes its own tick.
    for blk in nc.m.functions[0].blocks:
        for inst in blk.instructions:
            si = getattr(inst, "sync_info", None)
            if not si or not si.on_wait or len(si.on_wait) <= 1:
                continue
            own_sems = {u.id for u in (si.on_update or [])
                        if u.update_mode == "sem-inc"}
            tick = inst.bass_scheduled_tick
            kept = [w for w in si.on_wait
                    if not (w.id in own_sems and tick is not None
                            and w.wait_value < tick)]
            assert kept, f"{inst.name}: all waits dropped"
            si.on_wait = kept

    return nc


def _get_nc():
    global _CACHED_NC
    if _CACHED_NC is None:
        _CACHED_NC = _build_bass()
    return _CACHED_NC


# ----------------------------------------------------------------- entrypoint

def kernel(x, weights, weights_1, weights_2, _trace=False):
    x = np.asarray(x, dtype=np.float32)
    xf = x.reshape(B, DIM)
    # normalize rows on the host (packing-time math): the device then skips
    # the reciprocal/divide entirely and the signs contraction is final
    xf = xf / np.sqrt(np.sum(xf * xf, axis=1, keepdims=True))

    M = _build_circuit_matrix(weights)
    mre_pack = _pack_m(M.real.astype(np.float32)).astype(np.float16)
    mim_pack = _pack_m(M.imag.astype(np.float32)).astype(np.float16)
    sgn_pack = _pack_k_major(_signs()).astype(np.float16)

    in_maps = []
    for c in range(N_CORES):
        shard = xf[c * BS:(c + 1) * BS]              # (BS, DIM)
        xt = np.ascontiguousarray(shard.T)           # (DIM, BS)
        xt_pack = _pack_k_major(xt).astype(np.float16)  # (128, KT, BS)
        in_maps.append({
            "xt": xt_pack,
            "m_re": mre_pack,
            "m_im": mim_pack,
            "sgn": sgn_pack,
        })

    nc = _get_nc()
    res = run_bass_kernel_spmd(nc, in_maps, core_ids=list(range(N_CORES)),
                               trace=_trace)
    out = np.concatenate([r["outT"].T for r in res.results], axis=0)
    if _trace:
        kernel.last_exec_time_ns = res.exec_time_ns
        kernel.last_results = res
    return np.ascontiguousarray(out, dtype=np.float32)
